# revision 1
# baseline (speedup 1.0000x reference)
"""Bandpass biquad cascade (lowpass 3400Hz -> highpass 300Hz) on TRN2.

The two biquads are stable IIR filters (pole radii 0.43 / 0.92), so the
cascade's impulse response decays below fp32 noise within ~260 samples.
The filter is therefore computed exactly (to fp32 precision) as a
truncated FIR via block-Toeplitz matmuls on the tensor engine:

  y[k*128 + v] = sum_j sum_r W_j[r, v] * x[(k-j)*128 + r],
  W_j[r, v] = h_band[j*128 + v - r]   (J = 3 lag blocks)

Layout per core: 8 channels x 16 time-slices = 128 SBUF partitions, each
holding a contiguous 30000-sample slice (234 full chunks of 128 plus a
48-sample partial chunk, zero-padded). Per chunk:
PE transpose (time onto partitions) -> ACT copy to SBUF -> 3 fp32
matmuls with the transposed chunk as the *stationary* operand, which
makes the conv output land directly back in DMA-friendly layout
(partition = slice) -> DVE clamp-copy -> DMA out. Slice boundaries are
handled with a 2-chunk halo (zero for each channel's first slice,
matching the filter's zero initial state).
"""

import numpy as np

# ---------------- problem constants (hardcoded per contract) ----------------
B, C, T = 32, 2, 480000
N_CORES = 8
CH_PER_CORE = (B * C) // N_CORES  # 8 channels per core
NSLICE = 16                       # time-slices per channel
NPART = CH_PER_CORE * NSLICE      # 128 partitions (full SBUF width)
SLICE_T = T // NSLICE             # 30000
CHUNK = 128
CHUNKS = -(-SLICE_T // CHUNK)     # 235: 234 full + 1 partial (48 samples)
TAIL = SLICE_T - (CHUNKS - 1) * CHUNK  # 48 valid samples in the last chunk
J = 3                             # lag blocks: taps 0..383, worst-case cover 257
NTAPS = J * CHUNK
HALO = J - 1                      # halo chunks per slice
SG = 10                           # chunks per DMA strip
NSTRIPS = -(-CHUNKS // SG)        # 24 (last strip has 5 chunks)
GROUP = 4                         # output chunks per PSUM bank (512 fp32)
PIPE_LAG = 4                      # groups of PE-transpose lookahead before matmuls

LP = (0.22711797, 0.45423594, 0.22711797, -0.2766646, 0.18513647)
HP = (0.9200662, -1.8401324, 0.9200662, -1.8337326, 0.846532)


def _impulse(coeffs, n):
    b0, b1, b2, a1, a2 = (float(v) for v in coeffs)
    h = np.zeros(n)
    s1 = s2 = 0.0
    for t in range(n):
        xi = 1.0 if t == 0 else 0.0
        y = b0 * xi + s1
        s1 = b1 * xi - a1 * y + s2
        s2 = b2 * xi - a2 * y
        h[t] = y
    return h


def build_weights():
    """[128, J*128] fp32: column block j is W_j[r, v] = h[j*128 + v - r]."""
    h = np.convolve(_impulse(LP, NTAPS), _impulse(HP, NTAPS))[:NTAPS]
    idx = np.arange(CHUNK)
    blocks = []
    for j in range(J):
        tap = j * CHUNK + idx[None, :] - idx[:, None]  # [r, v]
        w = np.where((tap >= 0) & (tap < NTAPS), h[np.clip(tap, 0, NTAPS - 1)], 0.0)
        blocks.append(w)
    return np.concatenate(blocks, axis=1).astype(np.float32)


def build_halo_mask():
    """[120,120] fp32 diag: 1 everywhere except 0 on each channel's first
    slice (those slices have no predecessor -> zero initial state)."""
    m = np.ones(NPART, np.float32)
    m[:: NSLICE] = 0.0
    return np.diag(m).astype(np.float32)


def build_weights_bf16_tail():
    """Last lag block in bf16: |h[t]| <= 2.3e-5 for t >= 128, so bf16
    rounding of this block contributes < 5e-8 absolute output error."""
    import ml_dtypes
    w = build_weights()
    return w[:, (J - 1) * CHUNK :].astype(ml_dtypes.bfloat16)


# ---------------- walrus workaround ----------------
_CTRL_TYPES = ("InstDrain", "InstNoOp", "InstEventSemaphore")


def _split_excess_waits(nc, max_waits=1):
    """The nix walrus rejects instructions with too many sync waits (CTRL-type
    ops take only 1). Peel excess waits onto preceding same-engine NoOps."""
    import concourse.mybir as mybir

    for f in nc.m.functions:
        for blk in f.blocks:
            out = []
            changed = False
            for ins in blk.instructions:
                si = ins.sync_info
                ow = list(si.on_wait) if (si is not None and si.on_wait) else []
                lim = 1 if type(ins).__name__ in _CTRL_TYPES else max_waits
                if len(ow) > lim:
                    changed = True
                    k = 0
                    while len(ow) > lim:
                        head, ow = ow[:1], ow[1:]
                        out.append(
                            mybir.InstNoOp(
                                name=f"{ins.name}-waitsplit-{k}",
                                engine=ins.engine,
                                ins=[],
                                outs=[],
                                sync_info=mybir.SyncInfo(on_wait=head, on_update=[]),
                            )
                        )
                        k += 1
                    ins.sync_info = mybir.SyncInfo(
                        on_wait=ow,
                        on_update=list(si.on_update) if si.on_update else [],
                    )
                out.append(ins)
            if changed:
                blk.instructions = out


# ---------------- bass program ----------------
_CACHE = {}


def _build_bass():
    import concourse.bass as bass
    import concourse.mybir as mybir
    import concourse.tile as tile
    from concourse.masks import make_identity
    from contextlib import ExitStack

    fp32 = mybir.dt.float32
    bf16 = mybir.dt.bfloat16
    nc = bass.Bass()
    x = nc.dram_tensor("x", [CH_PER_CORE * T], fp32, kind="ExternalInput")
    w = nc.dram_tensor("w", [CHUNK, J * CHUNK], fp32, kind="ExternalInput")
    w2 = nc.dram_tensor("w2", [CHUNK, CHUNK], bf16, kind="ExternalInput")
    hm = nc.dram_tensor("hm", [NPART, NPART], fp32, kind="ExternalInput")
    y = nc.dram_tensor("y", [CH_PER_CORE * T], fp32, kind="ExternalOutput")

    # flat [ch*T + s*SLICE_T + t] == [(ch*NSLICE+s)*SLICE_T + t] since
    # NSLICE*SLICE_T == T; rows of this view are exactly the partitions.
    xv = x.rearrange("(p t) -> p t", p=NPART)
    yv = y.rearrange("(p t) -> p t", p=NPART)

    with tile.TileContext(nc) as tc, ExitStack() as ctx:
        const = ctx.enter_context(tc.tile_pool(name="const", bufs=1))
        in_pool = ctx.enter_context(tc.tile_pool(name="in", bufs=5))
        out_pool = ctx.enter_context(tc.tile_pool(name="out", bufs=3))
        xa_pool = ctx.enter_context(tc.tile_pool(name="xa", bufs=9))
        xab_pool = ctx.enter_context(tc.tile_pool(name="xab", bufs=9))
        pt_pool = ctx.enter_context(tc.tile_pool(name="pt", bufs=4, space="PSUM"))
        py_pool = ctx.enter_context(tc.tile_pool(name="py", bufs=4, space="PSUM"))

        in_tiles = {}

        def prefetch_strip(strip, eng=None, pieces=2):
            if strip not in in_tiles and strip < NSTRIPS:
                it = in_pool.tile([NPART, SG * CHUNK], fp32, name="in_strip")
                base = strip * SG * CHUNK
                nsamp = min(SLICE_T, (strip + 1) * SG * CHUNK) - base
                nchunk_cols = (min(CHUNKS, (strip + 1) * SG) - strip * SG) * CHUNK
                if nsamp < nchunk_cols:
                    # partial last chunk: zero-pad beyond the valid samples
                    nc.vector.memset(it[:, nsamp:nchunk_cols], 0.0)
                    pieces = 1
                step = nsamp // pieces
                for pc in range(pieces):
                    lo = pc * step
                    hi = nsamp if pc == pieces - 1 else (pc + 1) * step
                    (eng or nc.sync).dma_start(
                        it[:, lo:hi], xv[:, base + lo : base + hi]
                    )
                in_tiles[strip] = it

        # PE warmup: ~3.5us of dummy bf16 matmuls so the HAM clock-gate
        # opens (K=8/8) before the real transposes arrive
        wu = const.tile([CHUNK, 2 * CHUNK], bf16)
        nc.gpsimd.memset(wu[:], 0.0)
        wu_ps = pt_pool.tile([CHUNK, 512], fp32, name="pt_grp", tag="pt_grp")
        for _ in range(36):
            nc.tensor.matmul(
                wu_ps[:, :CHUNK], lhsT=wu[:, :CHUNK], rhs=wu[:, CHUNK:],
                start=True, stop=True,
            )
        prefetch_strip(0, eng=nc.scalar, pieces=5)
        prefetch_strip(1, eng=nc.scalar, pieces=2)
        for s0 in range(2, 5):
            prefetch_strip(s0)
        ident = const.tile([NPART, NPART], fp32)
        make_identity(nc, ident)
        wt = const.tile([CHUNK, J * CHUNK], fp32)
        nc.sync.dma_start(wt[:], w[:, :])
        wt2 = const.tile([CHUNK, CHUNK], bf16)
        nc.sync.dma_start(wt2[:], w2[:, :])
        hmt = const.tile([NPART, NPART], fp32)
        nc.sync.dma_start(hmt[:], hm[:, :])

        # halo: last HALO chunks of the previous slice; channel-start slices
        # get zero state, applied via the masked-diag transpose below
        halo = const.tile([NPART, HALO * CHUNK], fp32)
        nc.vector.memset(halo[:], 0.0)
        nc.sync.dma_start(
            halo[1:NPART, :],
            xv[0 : NPART - 1, SLICE_T - HALO * CHUNK : SLICE_T],
        )

        xa_of = {}        # chunk index -> (sbuf tile, column offset)
        out_tiles = {}
        group_sizes = []
        pos = 0
        while pos < CHUNKS:
            g = min(GROUP, CHUNKS - pos, SG - (pos % SG) if pos % SG else SG)
            # keep groups within a strip
            g = min(g, SG - (pos % SG))
            group_sizes.append((pos, g))
            pos += g

        def emit_halo_transposes():
            for hk in range(-HALO, 0):
                pt = pt_pool.tile([CHUNK, 512], fp32, name="pt_grp", tag="pt_grp")
                # transpose + zero channel-start columns in one op:
                # out = halo_chunk.T @ diag(mask)
                nc.tensor.matmul(
                    pt[:, :NPART],
                    lhsT=halo[:, (hk + HALO) * CHUNK : (hk + HALO + 1) * CHUNK],
                    rhs=hmt[:],
                    start=True,
                    stop=True,
                )
                xa = xa_pool.tile([CHUNK, GROUP, NPART], fp32, name="xa_t", tag="xa_t")
                nc.scalar.copy(xa[:, 0, :], pt[:, :NPART])
                xab = xab_pool.tile([CHUNK, GROUP, NPART], bf16, name="xab_t", tag="xab_t")
                nc.vector.tensor_copy(xab[:, 0, :], xa[:, 0, :])
                xa_of[hk] = (xa, xab, 0)

        def emit_transpose_group(gi):
            pos, g = group_sizes[gi]
            strip = pos // SG
            prefetch_strip(strip)
            it = in_tiles[strip]
            if pos + GROUP >= (strip + 1) * SG:
                prefetch_strip(strip + 1)
                prefetch_strip(strip + 2)
            pt = pt_pool.tile([CHUNK, 512], fp32, name="pt_grp", tag="pt_grp")
            for q in range(g):
                k = pos + q
                kl = k - strip * SG
                nc.tensor.transpose(
                    pt[:, q * CHUNK : q * CHUNK + NPART],
                    it[:, kl * CHUNK : (kl + 1) * CHUNK],
                    ident,
                )
            xa = xa_pool.tile([CHUNK, GROUP, NPART], fp32, name="xa_t", tag="xa_t")
            # one strided ACT copy for the whole group
            nc.scalar.copy(
                xa[:, :g, :],
                pt.rearrange("p (q c) -> p q c", c=CHUNK)[:, :g, :NPART],
            )
            xab = xab_pool.tile([CHUNK, GROUP, NPART], bf16, name="xab_t", tag="xab_t")
            nc.vector.tensor_copy(xab[:, :g, :], xa[:, :g, :])
            for q in range(g):
                xa_of[pos + q] = (xa, xab, q)

        def emit_matmul_group(gi):
            pos, g = group_sizes[gi]
            strip = pos // SG
            if strip not in out_tiles:
                out_tiles[strip] = out_pool.tile([NPART, SG * CHUNK], fp32, name="out_strip")
            ot = out_tiles[strip]
            py = py_pool.tile([NPART, 512], fp32, name="py_grp")
            # one fused fp32 matmul [W0|W1] per input chunk (shares the
            # stationary-operand load), plus one bf16 W2 matmul per chunk
            mms = []
            for i in range(pos - 1, pos + g):
                jlo = max(0, pos - i)
                jhi = min(1, pos + g - 1 - i)
                if jlo > jhi:
                    continue
                xa, _, off = xa_of[i]
                mms.append(
                    (
                        xa[:, off, :],
                        wt[:, jlo * CHUNK : (jhi + 1) * CHUNK],
                        (i + jlo - pos) * CHUNK,
                        (jhi - jlo + 1) * CHUNK,
                    )
                )
            for i in range(pos - 2, pos + g - 2):
                _, xab, off = xa_of[i]
                mms.append((xab[:, off, :], wt2[:], (i + 2 - pos) * CHUNK, CHUNK))
            for i_mm, (lhsT, rhs, col0, width) in enumerate(mms):
                nc.tensor.matmul(
                    py[:, col0 : col0 + width],
                    lhsT=lhsT,
                    rhs=rhs,
                    start=(i_mm == 0),
                    stop=(i_mm == len(mms) - 1),
                )
            kl0 = pos - strip * SG
            nc.vector.tensor_scalar(
                ot[:, kl0 * CHUNK : (kl0 + g) * CHUNK],
                py[:, : g * CHUNK],
                1.0,
                -1.0,
                mybir.AluOpType.min,
                mybir.AluOpType.max,
            )
            if strip == NSTRIPS - 1:
                # final strip: ship each group as soon as it drains,
                # clamped to the valid sample range of the partial chunk
                base = strip * SG * CHUNK
                lo = kl0 * CHUNK
                hi = min((kl0 + g) * CHUNK, SLICE_T - base)
                nc.sync.dma_start(yv[:, base + lo : base + hi], ot[:, lo:hi])
                if pos + g == CHUNKS:
                    del out_tiles[strip]
                    if strip in in_tiles:
                        del in_tiles[strip]
            elif kl0 + g == SG:
                nc.sync.dma_start(
                    yv[:, strip * SG * CHUNK : (strip + 1) * SG * CHUNK], ot[:]
                )
                del out_tiles[strip]
                if strip in in_tiles:
                    del in_tiles[strip]

        n_groups = len(group_sizes)
        for gi in range(n_groups + PIPE_LAG):
            if gi < n_groups:
                emit_transpose_group(gi)
            if gi < 8:
                # pad PE with dummy matmuls during the DMA ramp so the HAM
                # clock-gate stays open between the warmup burst and the
                # steady-state dense phase
                for _ in range(8):
                    nc.tensor.matmul(
                        wu_ps[:, :CHUNK], lhsT=wu[:, :CHUNK], rhs=wu[:, CHUNK:],
                        start=True, stop=True,
                    )
            if gi == PIPE_LAG - 1:
                emit_halo_transposes()
            if gi >= PIPE_LAG:
                emit_matmul_group(gi - PIPE_LAG)

    _split_excess_waits(nc)
    return nc


def _get_nc():
    if "nc" not in _CACHE:
        _CACHE["nc"] = _build_bass()
        _CACHE["w"] = build_weights()
        _CACHE["w2"] = build_weights_bf16_tail()
        _CACHE["hm"] = build_halo_mask()
    return _CACHE["nc"], _CACHE["w"], _CACHE["w2"], _CACHE["hm"]


def kernel(waveform: np.ndarray) -> np.ndarray:
    from concourse.bass_utils import run_bass_kernel_spmd

    nc, w, w2, hm = _get_nc()
    waveform = np.ascontiguousarray(waveform, dtype=np.float32)
    per_core = B // N_CORES  # batches per core
    in_maps = [
        {
            "x": waveform[i * per_core : (i + 1) * per_core].reshape(-1),
            "w": w,
            "w2": w2,
            "hm": hm,
        }
        for i in range(N_CORES)
    ]
    res = run_bass_kernel_spmd(nc, in_maps, core_ids=list(range(N_CORES)))
    out = np.concatenate(
        [r["y"].reshape(per_core, C, T) for r in res.results], axis=0
    )
    return out



# revision 5
# speedup vs baseline: 2.5124x; 2.5124x over previous
"""Bandpass biquad cascade (lowpass 3400Hz -> highpass 300Hz) on TRN2.

The two biquads are stable IIR filters (pole radii 0.43 / 0.92), so the
cascade's impulse response decays geometrically (|h[t]| <= 2.3e-5 for
t >= 128, against an output scale of ~0.34 and a 2e-2 relative error
budget). The filter is computed as a truncated FIR via block-Toeplitz
matmuls on the tensor engine:

  y[k*128 + v] = sum_j sum_r W_j[r, v] * x[(k-j)*128 + r],
  W_j[r, v] = h_band[j*128 + v - r]   (J = 2 lag blocks, >=129 taps)

Everything runs in fp16: the host converts the fp32 input to fp16 (and
the fp16 result back), halving HBM traffic, and fp16 matmuls/transposes
run at 1 PE cycle/row vs fp32's 4 (matmul) / 2 (transpose). End-to-end
quantization + truncation error is ~3e-4 relative, far inside the gate.

Layout per core: 8 channels x 16 time-slices = 128 SBUF partitions, each
holding a contiguous 30000-sample slice (234 full chunks of 128 plus a
48-sample partial chunk, zero-padded). Per chunk:
PE transpose (time onto partitions, fp16) -> ACT copy to SBUF -> fused
[W0|W1] fp16 matmul with the transposed chunk as the *stationary*
operand, which makes the conv output land directly back in DMA-friendly
layout (partition = slice) -> DVE clamp-copy -> DMA out. Slice
boundaries are handled with a 1-chunk halo loaded via a strided DMA that
skips each channel's first slice (those keep the memset zeros, matching
the filter's zero initial state).
"""

import numpy as np

# ---------------- problem constants (hardcoded per contract) ----------------
B, C, T = 32, 2, 480000
N_CORES = 8
CH_PER_CORE = (B * C) // N_CORES  # 8 channels per core
NSLICE = 16                       # time-slices per channel
NPART = CH_PER_CORE * NSLICE      # 128 partitions (full SBUF width)
SLICE_T = T // NSLICE             # 30000
CHUNK = 128
CHUNKS = -(-SLICE_T // CHUNK)     # 235: 234 full + 1 partial (48 samples)
TAIL = SLICE_T - (CHUNKS - 1) * CHUNK  # 48 valid samples in the last chunk
J = 2                             # lag blocks: taps 0..255, worst-case cover 129
NTAPS = J * CHUNK
HALO = J - 1                      # halo chunks per slice
SG = 10                           # chunks per DMA strip
NSTRIPS = -(-CHUNKS // SG)        # 24 (last strip has 5 chunks)
GROUP = 4                         # output chunks per PSUM bank (512 fp32)
PIPE_LAG = 4                      # groups of PE-transpose lookahead before matmuls

LP = (0.22711797, 0.45423594, 0.22711797, -0.2766646, 0.18513647)
HP = (0.9200662, -1.8401324, 0.9200662, -1.8337326, 0.846532)


def _impulse(coeffs, n):
    b0, b1, b2, a1, a2 = (float(v) for v in coeffs)
    h = np.zeros(n)
    s1 = s2 = 0.0
    for t in range(n):
        xi = 1.0 if t == 0 else 0.0
        y = b0 * xi + s1
        s1 = b1 * xi - a1 * y + s2
        s2 = b2 * xi - a2 * y
        h[t] = y
    return h


def build_weights():
    """[128, J*128] fp16: column block j is W_j[r, v] = h[j*128 + v - r]."""
    h = np.convolve(_impulse(LP, NTAPS), _impulse(HP, NTAPS))[:NTAPS]
    idx = np.arange(CHUNK)
    blocks = []
    for j in range(J):
        tap = j * CHUNK + idx[None, :] - idx[:, None]  # [r, v]
        w = np.where((tap >= 0) & (tap < NTAPS), h[np.clip(tap, 0, NTAPS - 1)], 0.0)
        blocks.append(w)
    return np.concatenate(blocks, axis=1).astype(np.float16)


# ---------------- walrus workaround ----------------
_CTRL_TYPES = ("InstDrain", "InstNoOp", "InstEventSemaphore")


def _split_excess_waits(nc, max_waits=1):
    """The nix walrus rejects instructions with too many sync waits (CTRL-type
    ops take only 1). Peel excess waits onto preceding same-engine NoOps."""
    import concourse.mybir as mybir

    for f in nc.m.functions:
        for blk in f.blocks:
            out = []
            changed = False
            for ins in blk.instructions:
                si = ins.sync_info
                ow = list(si.on_wait) if (si is not None and si.on_wait) else []
                lim = 1 if type(ins).__name__ in _CTRL_TYPES else max_waits
                if len(ow) > lim:
                    changed = True
                    k = 0
                    while len(ow) > lim:
                        head, ow = ow[:1], ow[1:]
                        out.append(
                            mybir.InstNoOp(
                                name=f"{ins.name}-waitsplit-{k}",
                                engine=ins.engine,
                                ins=[],
                                outs=[],
                                sync_info=mybir.SyncInfo(on_wait=head, on_update=[]),
                            )
                        )
                        k += 1
                    ins.sync_info = mybir.SyncInfo(
                        on_wait=ow,
                        on_update=list(si.on_update) if si.on_update else [],
                    )
                out.append(ins)
            if changed:
                blk.instructions = out


# ---------------- bass program ----------------
_CACHE = {}


def _build_bass():
    import concourse.bass as bass
    import concourse.mybir as mybir
    import concourse.tile as tile
    from concourse.masks import make_identity
    from contextlib import ExitStack

    fp32 = mybir.dt.float32
    fp16 = mybir.dt.float16
    bf16 = mybir.dt.bfloat16
    nc = bass.Bass()
    x = nc.dram_tensor("x", [CH_PER_CORE * T], fp16, kind="ExternalInput")
    w = nc.dram_tensor("w", [CHUNK, J * CHUNK], fp16, kind="ExternalInput")
    y = nc.dram_tensor("y", [CH_PER_CORE * T], fp16, kind="ExternalOutput")

    # flat [ch*T + s*SLICE_T + t] == [(ch*NSLICE+s)*SLICE_T + t] since
    # NSLICE*SLICE_T == T; rows of this view are exactly the partitions.
    xv = x.rearrange("(p t) -> p t", p=NPART)
    yv = y.rearrange("(p t) -> p t", p=NPART)

    with tile.TileContext(nc) as tc, ExitStack() as ctx:
        const = ctx.enter_context(tc.tile_pool(name="const", bufs=1))
        in_pool = ctx.enter_context(tc.tile_pool(name="in", bufs=5))
        out_pool = ctx.enter_context(tc.tile_pool(name="out", bufs=3))
        xa_pool = ctx.enter_context(tc.tile_pool(name="xa", bufs=9))
        pt_pool = ctx.enter_context(tc.tile_pool(name="pt", bufs=3, space="PSUM"))
        wu_pool = ctx.enter_context(tc.tile_pool(name="wu", bufs=1, space="PSUM"))
        py_pool = ctx.enter_context(tc.tile_pool(name="py", bufs=4, space="PSUM"))

        in_tiles = {}

        def prefetch_strip(strip, eng=None, pieces=2):
            if strip not in in_tiles and strip < NSTRIPS:
                it = in_pool.tile([NPART, SG * CHUNK], fp16, name="in_strip")
                base = strip * SG * CHUNK
                nsamp = min(SLICE_T, (strip + 1) * SG * CHUNK) - base
                nchunk_cols = (min(CHUNKS, (strip + 1) * SG) - strip * SG) * CHUNK
                if nsamp < nchunk_cols:
                    # partial last chunk: zero-pad beyond the valid samples
                    nc.vector.memset(it[:, nsamp:nchunk_cols], 0.0)
                    pieces = 1
                step = nsamp // pieces
                for pc in range(pieces):
                    lo = pc * step
                    hi = nsamp if pc == pieces - 1 else (pc + 1) * step
                    (eng or nc.sync).dma_start(
                        it[:, lo:hi], xv[:, base + lo : base + hi]
                    )
                in_tiles[strip] = it

        # PE warmup: ~3.5us of dummy bf16 matmuls so the HAM clock-gate
        # opens (K=8/8) before the real transposes arrive
        wu = const.tile([CHUNK, 2 * CHUNK], bf16)
        nc.gpsimd.memset(wu[:], 0.0)
        wu_ps = wu_pool.tile([CHUNK, 512], fp32, name="wu_ps", tag="wu_ps")
        for _ in range(36):
            nc.tensor.matmul(
                wu_ps[:, :CHUNK], lhsT=wu[:, :CHUNK], rhs=wu[:, CHUNK:],
                start=True, stop=True,
            )
        prefetch_strip(0, eng=nc.scalar, pieces=5)
        prefetch_strip(1, eng=nc.scalar, pieces=2)
        for s0 in range(2, 5):
            prefetch_strip(s0)
        ident = const.tile([NPART, NPART], fp16)
        make_identity(nc, ident)
        wt = const.tile([CHUNK, J * CHUNK], fp16)
        nc.sync.dma_start(wt[:], w[:, :])

        # halo: last chunk of the previous slice; channel-start slices keep
        # the memset zeros (no predecessor -> zero initial filter state).
        # One 2D DMA per channel so partition ranges stay contiguous.
        halo = const.tile([NPART, HALO * CHUNK], fp16)
        nc.vector.memset(halo[:], 0.0)
        for ch in range(CH_PER_CORE):
            p0 = ch * NSLICE
            nc.sync.dma_start(
                halo[p0 + 1 : p0 + NSLICE, :],
                xv[p0 : p0 + NSLICE - 1, SLICE_T - HALO * CHUNK : SLICE_T],
            )

        xa_of = {}        # chunk index -> (sbuf tile, column offset)
        out_tiles = {}
        group_sizes = []
        pos = 0
        while pos < CHUNKS:
            g = min(GROUP, CHUNKS - pos, SG - (pos % SG) if pos % SG else SG)
            # keep groups within a strip
            g = min(g, SG - (pos % SG))
            group_sizes.append((pos, g))
            pos += g

        def emit_halo_transposes():
            for hk in range(-HALO, 0):
                pt = pt_pool.tile([CHUNK, 1024], fp16, name="pt_grp", tag="pt_grp")
                nc.tensor.transpose(
                    pt[:, :NPART],
                    halo[:, (hk + HALO) * CHUNK : (hk + HALO + 1) * CHUNK],
                    ident,
                )
                xa = xa_pool.tile([CHUNK, GROUP, NPART], fp16, name="xa_t", tag="xa_t")
                nc.scalar.copy(xa[:, 0, :], pt[:, :NPART])
                xa_of[hk] = (xa, 0)

        def emit_transpose_group(gi):
            pos, g = group_sizes[gi]
            strip = pos // SG
            prefetch_strip(strip)
            it = in_tiles[strip]
            if pos + GROUP >= (strip + 1) * SG:
                prefetch_strip(strip + 1)
                prefetch_strip(strip + 2)
            pt = pt_pool.tile([CHUNK, 1024], fp16, name="pt_grp", tag="pt_grp")
            for q in range(g):
                k = pos + q
                kl = k - strip * SG
                nc.tensor.transpose(
                    pt[:, q * CHUNK : q * CHUNK + NPART],
                    it[:, kl * CHUNK : (kl + 1) * CHUNK],
                    ident,
                )
            xa = xa_pool.tile([CHUNK, GROUP, NPART], fp16, name="xa_t", tag="xa_t")
            # one strided ACT copy for the whole group
            nc.scalar.copy(
                xa[:, :g, :],
                pt[:, : g * CHUNK].rearrange("p (q c) -> p q c", c=CHUNK)[:, :g, :NPART],
            )
            for q in range(g):
                xa_of[pos + q] = (xa, q)

        def emit_matmul_group(gi):
            pos, g = group_sizes[gi]
            strip = pos // SG
            if strip not in out_tiles:
                out_tiles[strip] = out_pool.tile([NPART, SG * CHUNK], fp16, name="out_strip")
            ot = out_tiles[strip]
            py = py_pool.tile([NPART, 512], fp32, name="py_grp")
            # one fused fp16 matmul [W0|W1] per input chunk (shares the
            # stationary-operand load)
            mms = []
            for i in range(pos - 1, pos + g):
                jlo = max(0, pos - i)
                jhi = min(1, pos + g - 1 - i)
                if jlo > jhi:
                    continue
                xa, off = xa_of[i]
                mms.append(
                    (
                        xa[:, off, :],
                        wt[:, jlo * CHUNK : (jhi + 1) * CHUNK],
                        (i + jlo - pos) * CHUNK,
                        (jhi - jlo + 1) * CHUNK,
                    )
                )
            for i_mm, (lhsT, rhs, col0, width) in enumerate(mms):
                nc.tensor.matmul(
                    py[:, col0 : col0 + width],
                    lhsT=lhsT,
                    rhs=rhs,
                    start=(i_mm == 0),
                    stop=(i_mm == len(mms) - 1),
                )
            kl0 = pos - strip * SG
            nc.vector.tensor_scalar(
                ot[:, kl0 * CHUNK : (kl0 + g) * CHUNK],
                py[:, : g * CHUNK],
                1.0,
                -1.0,
                mybir.AluOpType.min,
                mybir.AluOpType.max,
            )
            if strip == NSTRIPS - 1:
                # final strip: ship each group as soon as it drains,
                # clamped to the valid sample range of the partial chunk
                base = strip * SG * CHUNK
                lo = kl0 * CHUNK
                hi = min((kl0 + g) * CHUNK, SLICE_T - base)
                nc.gpsimd.dma_start(yv[:, base + lo : base + hi], ot[:, lo:hi])
                if pos + g == CHUNKS:
                    del out_tiles[strip]
                    if strip in in_tiles:
                        del in_tiles[strip]
            elif kl0 + g == SG:
                nc.gpsimd.dma_start(
                    yv[:, strip * SG * CHUNK : (strip + 1) * SG * CHUNK], ot[:]
                )
                del out_tiles[strip]
                if strip in in_tiles:
                    del in_tiles[strip]

        n_groups = len(group_sizes)
        for gi in range(n_groups + PIPE_LAG):
            if gi < n_groups:
                emit_transpose_group(gi)
            if gi < 8:
                # pad PE with dummy matmuls during the DMA ramp so the HAM
                # clock-gate stays open between the warmup burst and the
                # steady-state dense phase
                for _ in range(8):
                    nc.tensor.matmul(
                        wu_ps[:, :CHUNK], lhsT=wu[:, :CHUNK], rhs=wu[:, CHUNK:],
                        start=True, stop=True,
                    )
            if gi == PIPE_LAG - 1:
                emit_halo_transposes()
            if gi >= PIPE_LAG:
                emit_matmul_group(gi - PIPE_LAG)

    _split_excess_waits(nc)
    return nc


def _get_nc():
    if "nc" not in _CACHE:
        _CACHE["nc"] = _build_bass()
        _CACHE["w"] = build_weights()
    return _CACHE["nc"], _CACHE["w"]


def make_in_maps(waveform_f16: np.ndarray):
    """waveform_f16: [B, C, T] np.float16, C-contiguous."""
    _, w = _get_nc()
    per_core = B // N_CORES
    return [
        {
            "x": waveform_f16[i * per_core : (i + 1) * per_core].reshape(-1),
            "w": w,
        }
        for i in range(N_CORES)
    ]


def kernel(waveform: np.ndarray) -> np.ndarray:
    from concourse.bass_utils import run_bass_kernel_spmd

    nc, _ = _get_nc()
    xf16 = np.ascontiguousarray(waveform, dtype=np.float16)
    in_maps = make_in_maps(xf16)
    res = run_bass_kernel_spmd(nc, in_maps, core_ids=list(range(N_CORES)))
    per_core = B // N_CORES
    out = np.concatenate(
        [r["y"].reshape(per_core, C, T).astype(np.float32) for r in res.results],
        axis=0,
    )
    return out
